# revision 9
# baseline (speedup 1.0000x reference)
"""CoarseToFine gather+proj+merge kernel for 8 Trainium2 NeuronCores.

Reference computation (per match i of M, for two branches):
  window = 5x5 patch of fine map (stride-4 grid, pad 2), flattened
           CHANNEL-major then re-read as [25, 128] (torch-unfold + plain
           reshape => "scrambled" (c,k)->(a,d) relabeling)
  bias   = coarse[b, l] @ Wcomb.T + bcomb          (folded proj+merge1)
  out    = window_scrambled @ Wmerge2.T + bias     -> [25, 128]

Sharding strategy: shard by MATCH.  The 2*M = 4096 items are split
evenly, 512 per core, and each core's input shard is exactly its
matches' data: the 5x5x128 fine windows (host im2col -- pure data
relayout of the unfold -- stored contraction-major [d, (chunk, a, m)]
bf16) and its matches' coarse rows (stored [k, item] bf16).  Weights
are folded on host (Wm2.T, Wcomb = Wm1 @ Wproj, bcomb) and replicated.
All model arithmetic runs on device in bf16 with fp32 PSUM:

  bias[o, m]  = Wcomb . coarse          (2 accumulating matmuls + bcomb)
  per chunk of 128 items (4 chunks):
    DMA window shard -> ts[d, (a, m)]
    merge matmuls vs folded Wmerge2.T -> psum[o, (a, m)]  (4+3 per chunk
    into one 4-bank and one 3-bank PSUM tile, ping-ponged)
    + bias broadcast over a (DVE add for the 2048-col half; Act copy +
    Pool in-place add for the 1152-col half) -> bf16 merged
    -> o-major bf16 DMA out (two halves); host reorders to match order.
"""

import os
import numpy as np

WINDOW = 5
C = 128        # fine channels
HO, WO = 60, 80            # coarse grid
L = 4800                   # coarse positions
DC = 256                   # coarse dim
B = 2
M = 2048                   # matches per branch
CAP = 512                  # items per core (2*M / 8 exactly)
GC = 128                   # items per chunk
NCHUNK = CAP // GC         # 4
QD = 25 * GC               # window cols per chunk (a, m) = 3200
ACOLS = 2048               # A-half cols (a-blocks 0..15)
BCOLS = QD - ACOLS         # B-half cols (a-blocks 16..24) = 1152


# --------------------------------------------------------------------------
# sync-wait legalization: this walrus build accepts only ONE sync wait per
# instruction; overflow waits move to NOPs inserted just before, same engine.
def _split_sync_waits(nc, mybir, max_waits=1):
    for fn in nc.m.functions:
        for blk in fn.blocks:
            new_insts = []
            for inst in blk.instructions:
                si = getattr(inst, "sync_info", None)
                waits = list(si.on_wait) if si is not None and si.on_wait else []
                if len(waits) > max_waits:
                    for wt in waits[:-max_waits]:
                        nop = mybir.InstNoOp(
                            name=nc.get_next_instruction_name(),
                            engine=inst.engine,
                            ins=[],
                            outs=[],
                            sync_info=mybir.SyncInfo(on_wait=[wt], on_update=[]),
                        )
                        nc.register_instruction(nop)
                        new_insts.append(nop)
                    si.on_wait = waits[-max_waits:]
                new_insts.append(inst)
            blk.instructions = new_insts
    return nc


# --------------------------------------------------------------------------
def _build_program():
    import concourse.bacc as bacc
    import concourse.mybir as mybir
    import concourse.tile as tile

    dt = mybir.dt

    nc = bacc.Bacc("TRN2", target_bir_lowering=False, debug=False, num_devices=8)

    tsd = nc.dram_tensor("tsd", [128, NCHUNK * QD], dt.bfloat16, kind="ExternalInput").ap()
    ctd = nc.dram_tensor("ctd", [128, 2 * CAP], dt.bfloat16, kind="ExternalInput").ap()
    # wts: wm2t | wcta | wctb  (each [128, 128])
    wts = nc.dram_tensor("wts", [128, 384], dt.bfloat16, kind="ExternalInput").ap()
    bcomb = nc.dram_tensor("bcomb", [128], dt.float32, kind="ExternalInput").ap()
    out = nc.dram_tensor("out", [128 * CAP * 25], dt.bfloat16, kind="ExternalOutput").ap()
    outv = out.rearrange("(o q) -> o q", o=128)

    with tile.TileContext(nc) as tc:
        with (
            tc.tile_pool(name="const", bufs=1) as cpool,
            tc.tile_pool(name="ts", bufs=3) as tspool,
            tc.tile_pool(name="mg", bufs=2) as mpool,
            tc.tile_pool(name="psa", bufs=1, space="PSUM") as psa,
            tc.tile_pool(name="psb", bufs=1, space="PSUM") as psb,
            tc.tile_pool(name="psc", bufs=1, space="PSUM") as psc,
        ):
            wts_sb = cpool.tile([128, 384], dt.bfloat16)
            bc_sb = cpool.tile([128, 1], dt.float32)
            ct_sb = cpool.tile([128, 2 * CAP], dt.bfloat16)
            bias_sb = cpool.tile([128, CAP], dt.float32)

            # ts0 first: it gates the first merge matmul
            tss = []
            ts0 = tspool.tile([128, QD], dt.bfloat16, tag="ts")
            nc.sync.dma_start(ts0[:], tsd[:, 0:QD])
            tss.append(ts0)
            nc.sync.dma_start(wts_sb[:], wts[:])
            nc.sync.dma_start(ct_sb[:], ctd[:])
            for kc in (1, 2):
                ts = tspool.tile([128, QD], dt.bfloat16, tag="ts")
                nc.sync.dma_start(ts[:], tsd[:, kc * QD:(kc + 1) * QD])
                tss.append(ts)
            nc.sync.dma_start(bc_sb[:], bcomb[:].unsqueeze(1))
            wm2_sb = wts_sb[:, 0:128]
            wca_sb = wts_sb[:, 128:256]
            wcb_sb = wts_sb[:, 256:384]

            # bias[o, item] = Wcomb[o, :] . coarse[item, :] + bcomb[o]
            bps = psc.tile([128, CAP], dt.float32, space="PSUM", tag="b")
            nc.tensor.matmul(bps[:], lhsT=wca_sb, rhs=ct_sb[:, 0:CAP],
                             start=True, stop=False)
            nc.tensor.matmul(bps[:], lhsT=wcb_sb, rhs=ct_sb[:, CAP:2 * CAP],
                             start=False, stop=True)
            nc.vector.tensor_scalar_add(bias_sb[:], bps[:], bc_sb[:])

            for kc in range(NCHUNK):
                if kc >= 3:
                    ts = tspool.tile([128, QD], dt.bfloat16, tag="ts")
                    nc.sync.dma_start(ts[:], tsd[:, kc * QD:(kc + 1) * QD])
                else:
                    ts = tss[kc]
                merged = mpool.tile([128, QD], dt.bfloat16, tag="mg")
                bias_kc = bias_sb[:, kc * GC:(kc + 1) * GC]

                # A half: a-blocks 0..15 into one 4-bank PSUM tile
                mma = psa.tile([128, ACOLS], dt.float32, space="PSUM", tag="a")
                for g in range(4):
                    nc.tensor.matmul(mma[:, g * 512:(g + 1) * 512], lhsT=wm2_sb,
                                     rhs=ts[:, g * 512:(g + 1) * 512],
                                     start=True, stop=True)
                # split the A add in two so the next chunk's matmuls can
                # reclaim PSUM banks as soon as each half is drained
                for hh in range(2):
                    c0, c1 = hh * 1024, (hh + 1) * 1024
                    nc.vector.tensor_add(
                        merged[:, c0:c1].rearrange("p (a m) -> p a m", m=GC),
                        mma[:, c0:c1].rearrange("p (a m) -> p a m", m=GC),
                        bias_kc.unsqueeze(1).broadcast_to([128, 8, GC]),
                    )
                nc.sync.dma_start(outv[:, kc * QD:kc * QD + ACOLS], merged[:, 0:ACOLS])

                # B half: a-blocks 16..24 into a 3-bank PSUM tile;
                # Act copies PSUM->SBUF, Pool adds the bias in place.
                # Last chunk: DVE does the add (shorter path -> earlier tail).
                mmb = psb.tile([128, BCOLS], dt.float32, space="PSUM", tag="b")
                for g in range(3):
                    c0 = g * 512
                    c1 = min(BCOLS, c0 + 512)
                    nc.tensor.matmul(mmb[:, c0:c1], lhsT=wm2_sb,
                                     rhs=ts[:, ACOLS + c0:ACOLS + c1],
                                     start=True, stop=True)
                if kc < NCHUNK - 1:
                    nc.scalar.copy(merged[:, ACOLS:QD], mmb[:])
                    nc.gpsimd.tensor_add(
                        merged[:, ACOLS:QD].rearrange("p (a m) -> p a m", m=GC),
                        merged[:, ACOLS:QD].rearrange("p (a m) -> p a m", m=GC),
                        bias_kc.unsqueeze(1).broadcast_to([128, BCOLS // GC, GC]),
                    )
                else:
                    nc.vector.tensor_add(
                        merged[:, ACOLS:QD].rearrange("p (a m) -> p a m", m=GC),
                        mmb[:].rearrange("p (a m) -> p a m", m=GC),
                        bias_kc.unsqueeze(1).broadcast_to([128, BCOLS // GC, GC]),
                    )
                nc.sync.dma_start(outv[:, kc * QD + ACOLS:(kc + 1) * QD],
                                  merged[:, ACOLS:QD])

    nc.compile()
    import concourse.mybir as mybir
    _split_sync_waits(nc, mybir)
    return nc


# --------------------------------------------------------------------------
def _host_prep(inputs):
    import ml_dtypes
    bf16 = ml_dtypes.bfloat16

    f0 = np.asarray(inputs["feat_f0"], np.float32)
    f1 = np.asarray(inputs["feat_f1"], np.float32)
    c0 = np.asarray(inputs["feat_c0"], np.float32)
    c1 = np.asarray(inputs["feat_c1"], np.float32)
    b_ids = np.asarray(inputs["b_ids"]).astype(np.int64)
    l_ids = np.asarray(inputs["l_ids"]).astype(np.int64)
    s_ids = np.asarray(inputs["s_ids"]).astype(np.int64)
    wproj = np.asarray(inputs["W_proj"], np.float32)
    bproj = np.asarray(inputs["b_proj"], np.float32)
    wmerge = np.asarray(inputs["W_merge"], np.float32)
    bmerge = np.asarray(inputs["b_merge"], np.float32)

    # folded weights: merged = [c_proj | window] @ Wmerge.T + bmerge
    #   window part:  Wm2 = Wmerge[:, 128:]        (device: lhsT = Wm2.T)
    #   coarse part:  Wcomb = Wm1 @ Wproj, bcomb = Wm1 @ bproj + bmerge
    wm1, wm2 = wmerge[:, :128], wmerge[:, 128:]
    wcomb = wm1 @ wproj
    wts = np.concatenate(
        [wm2.T, wcomb[:, :128].T, wcomb[:, 128:].T], axis=1)
    wts = np.ascontiguousarray(wts).astype(bf16)
    bcv = (wm1 @ bproj + bmerge).astype(np.float32)

    # item shards: branch-major, original match order; core i owns
    # [512i, 512i+512).  Window extraction (the unfold) is pure relayout.
    b_all = np.concatenate([b_ids, b_ids])
    id_all = np.concatenate([l_ids, s_ids])
    h = (id_all // WO) * 4
    w = (id_all % WO) * 4
    fpad = np.stack([
        np.pad(f, ((0, 0), (0, 0), (2, 2), (2, 2))) for f in (f0, f1)
    ])                                              # [2, B, C, 244, 324]
    fpad = fpad.reshape(2 * B, C, 244, 324)
    mapid = np.repeat(np.arange(2), M) * B + b_all
    ki = np.arange(WINDOW)
    # windows[item, c, ki, kj] -> q = c*25 + ki*5 + kj (the torch scramble)
    wins = fpad[mapid[:, None, None, None],
                np.arange(C)[None, :, None, None],
                (h[:, None] + ki[None, :])[:, None, :, None],
                (w[:, None] + ki[None, :])[:, None, None, :]]
    wq = wins.reshape(2 * M, 25 * C).astype(bf16)   # [item, q]

    # coarse rows, pre-transposed to [k, item] per core
    cf = np.stack([c0, c1]).reshape(2 * B, L, DC)
    crows = cf[mapid, id_all].astype(bf16)          # [item, 256]

    in_maps = []
    for core in range(8):
        sl = slice(core * CAP, (core + 1) * CAP)
        # [d, (chunk, a, m)]
        tsd = np.ascontiguousarray(
            wq[sl].reshape(NCHUNK, GC, 25, 128).transpose(3, 0, 2, 1)
        ).reshape(128, NCHUNK * QD)
        # [k, (kchunk, item)]
        ctd = np.ascontiguousarray(
            crows[sl].reshape(CAP, 2, 128).transpose(2, 1, 0)
        ).reshape(128, 2 * CAP)
        in_maps.append({
            "tsd": tsd,
            "ctd": ctd,
            "wts": wts,
            "bcomb": bcv,
        })
    return in_maps


def _assemble(results):
    full = np.empty((2 * M, 25, 128), np.float32)
    for core, res in enumerate(results):
        og = np.asarray(res["out"]).reshape(128, NCHUNK, 25, GC)
        full[core * CAP:(core + 1) * CAP] = (
            og.transpose(1, 3, 2, 0).reshape(CAP, 25, 128).astype(np.float32))
    return full[:M], full[M:]


def _install_ntff_shim():
    """This image lacks ``antenv.axon_hooks``; recreate it so bass_utils'
    trace path can drive NTFF profiling via the axon PJRT .so."""
    import sys, types
    if "antenv.axon_hooks" in sys.modules:
        return
    import antenv  # noqa: F401
    mod = types.ModuleType("antenv.axon_hooks")
    mod._hook = None
    mod.set_axon_ntff_profile_hook = lambda h: setattr(mod, "_hook", h)
    mod.get_axon_ntff_profile_hook = lambda: mod._hook
    sys.modules["antenv.axon_hooks"] = mod
    try:
        from trn_agent_boot.trn_boot import _ntff_profile_via_ctypes
        mod._hook = _ntff_profile_via_ctypes("/opt/axon/libaxon_pjrt.so")
    except Exception:
        pass


def kernel(**inputs):
    from concourse import bass_utils

    in_maps = _host_prep(inputs)
    nc = _build_program()

    if os.environ.get("TRNK_SIM"):
        from concourse.bass_interp import CoreSim
        results = []
        ncore = int(os.environ.get("TRNK_SIM_CORES", "8"))
        for c in range(8):
            if c < ncore:
                sim = CoreSim(nc, trace=False)
                for name, val in in_maps[c].items():
                    sim.tensor(name)[:] = val
                sim.simulate()
                results.append({"out": np.array(sim.tensor("out"))})
            else:
                results.append({"out": np.zeros(128 * CAP * 25, np.float32)})
        return _assemble(results)

    trace = bool(os.environ.get("TRNK_TRACE"))
    kw = {}
    if trace:
        _install_ntff_shim()
        kw = dict(trace=True, trace_cores=list(range(8)))
    res = bass_utils.run_bass_kernel_spmd(nc, in_maps, core_ids=list(range(8)), **kw)
    if trace and res.exec_time_ns is not None:
        kernel.last_exec_time_ns = res.exec_time_ns
        kernel.last_mean_exec_time_ns = res.mean_exec_time_ns
        if res.instructions_and_trace:
            kernel.last_trace_path = res.instructions_and_trace[1]
    return _assemble(res.results)


kernel.last_exec_time_ns = None
kernel.last_mean_exec_time_ns = None
kernel.last_trace_path = None


# revision 11
# speedup vs baseline: 1.0795x; 1.0795x over previous
"""CoarseToFine gather+proj+merge kernel for 8 Trainium2 NeuronCores.

Reference computation (per match i of M, for two branches):
  window = 5x5 patch of fine map (stride-4 grid, pad 2), flattened
           CHANNEL-major then re-read as [25, 128] (torch-unfold + plain
           reshape => "scrambled" (c,k)->(a,d) relabeling)
  bias   = coarse[b, l] @ Wcomb.T + bcomb          (folded proj+merge1)
  out    = window_scrambled @ Wmerge2.T + bias     -> [25, 128]

Sharding strategy: shard by MATCH.  The 2*M = 4096 items are split
evenly, 512 per core, and each core's input shard is exactly its
matches' data: the 5x5x128 fine windows (host im2col -- pure data
relayout of the unfold -- stored contraction-major [d, (chunk, a, m)]
bf16) and its matches' coarse rows (stored [k, item] bf16).  Weights
are folded on host (Wm2.T, Wcomb = Wm1 @ Wproj, bcomb) and replicated.
All model arithmetic runs on device in bf16 with fp32 PSUM:

  bias[o, m]  = Wcomb . coarse          (2 accumulating matmuls + bcomb)
  per chunk of 128 items (4 chunks):
    DMA window shard -> ts[d, (a, m)]
    merge matmuls vs folded Wmerge2.T -> psum[o, (a, m)]  (4+3 per chunk
    into one 4-bank and one 3-bank PSUM tile, ping-ponged)
    + bias broadcast over a (DVE add for the 2048-col half; Act copy +
    Pool in-place add for the 1152-col half) -> bf16 merged
    -> o-major bf16 DMA out (two halves); host reorders to match order.
"""

import os
import numpy as np

WINDOW = 5
C = 128        # fine channels
HO, WO = 60, 80            # coarse grid
L = 4800                   # coarse positions
DC = 256                   # coarse dim
B = 2
M = 2048                   # matches per branch
CAP = 512                  # items per core (2*M / 8 exactly)
GC = 128                   # items per chunk
NCHUNK = CAP // GC         # 4
QD = 25 * GC               # window cols per chunk (a, m) = 3200
ACOLS = 2048               # A-half cols (a-blocks 0..15)
BCOLS = QD - ACOLS         # B-half cols (a-blocks 16..24) = 1152


# --------------------------------------------------------------------------
# sync-wait legalization: this walrus build accepts only ONE sync wait per
# instruction; overflow waits move to NOPs inserted just before, same engine.
def _split_sync_waits(nc, mybir, max_waits=1):
    for fn in nc.m.functions:
        for blk in fn.blocks:
            new_insts = []
            for inst in blk.instructions:
                si = getattr(inst, "sync_info", None)
                waits = list(si.on_wait) if si is not None and si.on_wait else []
                if len(waits) > max_waits:
                    for wt in waits[:-max_waits]:
                        nop = mybir.InstNoOp(
                            name=nc.get_next_instruction_name(),
                            engine=inst.engine,
                            ins=[],
                            outs=[],
                            sync_info=mybir.SyncInfo(on_wait=[wt], on_update=[]),
                        )
                        nc.register_instruction(nop)
                        new_insts.append(nop)
                    si.on_wait = waits[-max_waits:]
                new_insts.append(inst)
            blk.instructions = new_insts
    return nc


# --------------------------------------------------------------------------
def _build_program():
    import concourse.bacc as bacc
    import concourse.mybir as mybir
    import concourse.tile as tile

    dt = mybir.dt

    nc = bacc.Bacc("TRN2", target_bir_lowering=False, debug=False, num_devices=8)

    tsd = nc.dram_tensor("tsd", [128, NCHUNK * QD], dt.bfloat16, kind="ExternalInput").ap()
    ctd = nc.dram_tensor("ctd", [128, 2 * CAP], dt.bfloat16, kind="ExternalInput").ap()
    # wts: wm2t | wcta | wctb  (each [128, 128])
    wts = nc.dram_tensor("wts", [128, 384], dt.bfloat16, kind="ExternalInput").ap()
    bcomb = nc.dram_tensor("bcomb", [128], dt.float32, kind="ExternalInput").ap()
    out = nc.dram_tensor("out", [128 * CAP * 25], dt.bfloat16, kind="ExternalOutput").ap()
    outv = out.rearrange("(o q) -> o q", o=128)

    with tile.TileContext(nc) as tc:
        with (
            tc.tile_pool(name="const", bufs=1) as cpool,
            tc.tile_pool(name="ts", bufs=4) as tspool,
            tc.tile_pool(name="mg", bufs=4) as mpool,
            tc.tile_pool(name="psa", bufs=1, space="PSUM") as psa,
            tc.tile_pool(name="psb", bufs=1, space="PSUM") as psb,
            tc.tile_pool(name="psc", bufs=1, space="PSUM") as psc,
        ):
            wts_sb = cpool.tile([128, 384], dt.bfloat16)
            bc_sb = cpool.tile([128, 1], dt.float32)
            ct_sb = cpool.tile([128, 2 * CAP], dt.bfloat16)
            bias_sb = cpool.tile([128, CAP], dt.float32)

            # bias inputs first (they gate the adds), then the window shards
            nc.sync.dma_start(wts_sb[:], wts[:])
            nc.sync.dma_start(ct_sb[:], ctd[:])
            nc.sync.dma_start(bc_sb[:], bcomb[:].unsqueeze(1))
            tss = []
            for kc in range(NCHUNK):
                ts = tspool.tile([128, QD], dt.bfloat16, tag="ts")
                nc.sync.dma_start(ts[:], tsd[:, kc * QD:(kc + 1) * QD])
                tss.append(ts)
            wm2_sb = wts_sb[:, 0:128]
            wca_sb = wts_sb[:, 128:256]
            wcb_sb = wts_sb[:, 256:384]

            # bias[o, item] = Wcomb[o, :] . coarse[item, :] + bcomb[o]
            bps = psc.tile([128, CAP], dt.float32, space="PSUM", tag="b")
            nc.tensor.matmul(bps[:], lhsT=wca_sb, rhs=ct_sb[:, 0:CAP],
                             start=True, stop=False)
            nc.tensor.matmul(bps[:], lhsT=wcb_sb, rhs=ct_sb[:, CAP:2 * CAP],
                             start=False, stop=True)
            nc.vector.tensor_scalar_add(bias_sb[:], bps[:], bc_sb[:])

            for kc in range(NCHUNK):
                ts = tss[kc]
                merged = mpool.tile([128, QD], dt.bfloat16, tag="mg")
                bias_kc = bias_sb[:, kc * GC:(kc + 1) * GC]

                # A half: a-blocks 0..15 into one 4-bank PSUM tile
                mma = psa.tile([128, ACOLS], dt.float32, space="PSUM", tag="a")
                for g in range(4):
                    nc.tensor.matmul(mma[:, g * 512:(g + 1) * 512], lhsT=wm2_sb,
                                     rhs=ts[:, g * 512:(g + 1) * 512],
                                     start=True, stop=True)
                # split the A add in two so the next chunk's matmuls can
                # reclaim PSUM banks as soon as each half is drained
                for hh in range(2):
                    c0, c1 = hh * 1024, (hh + 1) * 1024
                    nc.vector.tensor_add(
                        merged[:, c0:c1].rearrange("p (a m) -> p a m", m=GC),
                        mma[:, c0:c1].rearrange("p (a m) -> p a m", m=GC),
                        bias_kc.unsqueeze(1).broadcast_to([128, 8, GC]),
                    )
                nc.sync.dma_start(outv[:, kc * QD:kc * QD + ACOLS], merged[:, 0:ACOLS])

                # B half: a-blocks 16..24 into a 3-bank PSUM tile;
                # Act copies PSUM->SBUF, Pool adds the bias in place.
                # Last chunk: DVE does the add (shorter path -> earlier tail).
                mmb = psb.tile([128, BCOLS], dt.float32, space="PSUM", tag="b")
                for g in range(3):
                    c0 = g * 512
                    c1 = min(BCOLS, c0 + 512)
                    nc.tensor.matmul(mmb[:, c0:c1], lhsT=wm2_sb,
                                     rhs=ts[:, ACOLS + c0:ACOLS + c1],
                                     start=True, stop=True)
                if kc < NCHUNK - 1:
                    nc.scalar.copy(merged[:, ACOLS:QD], mmb[:])
                    nc.gpsimd.tensor_add(
                        merged[:, ACOLS:QD].rearrange("p (a m) -> p a m", m=GC),
                        merged[:, ACOLS:QD].rearrange("p (a m) -> p a m", m=GC),
                        bias_kc.unsqueeze(1).broadcast_to([128, BCOLS // GC, GC]),
                    )
                else:
                    nc.vector.tensor_add(
                        merged[:, ACOLS:QD].rearrange("p (a m) -> p a m", m=GC),
                        mmb[:].rearrange("p (a m) -> p a m", m=GC),
                        bias_kc.unsqueeze(1).broadcast_to([128, BCOLS // GC, GC]),
                    )
                nc.sync.dma_start(outv[:, kc * QD + ACOLS:(kc + 1) * QD],
                                  merged[:, ACOLS:QD])

    nc.compile()
    import concourse.mybir as mybir
    _split_sync_waits(nc, mybir)
    return nc


# --------------------------------------------------------------------------
def _host_prep(inputs):
    import ml_dtypes
    bf16 = ml_dtypes.bfloat16

    f0 = np.asarray(inputs["feat_f0"], np.float32)
    f1 = np.asarray(inputs["feat_f1"], np.float32)
    c0 = np.asarray(inputs["feat_c0"], np.float32)
    c1 = np.asarray(inputs["feat_c1"], np.float32)
    b_ids = np.asarray(inputs["b_ids"]).astype(np.int64)
    l_ids = np.asarray(inputs["l_ids"]).astype(np.int64)
    s_ids = np.asarray(inputs["s_ids"]).astype(np.int64)
    wproj = np.asarray(inputs["W_proj"], np.float32)
    bproj = np.asarray(inputs["b_proj"], np.float32)
    wmerge = np.asarray(inputs["W_merge"], np.float32)
    bmerge = np.asarray(inputs["b_merge"], np.float32)

    # folded weights: merged = [c_proj | window] @ Wmerge.T + bmerge
    #   window part:  Wm2 = Wmerge[:, 128:]        (device: lhsT = Wm2.T)
    #   coarse part:  Wcomb = Wm1 @ Wproj, bcomb = Wm1 @ bproj + bmerge
    wm1, wm2 = wmerge[:, :128], wmerge[:, 128:]
    wcomb = wm1 @ wproj
    wts = np.concatenate(
        [wm2.T, wcomb[:, :128].T, wcomb[:, 128:].T], axis=1)
    wts = np.ascontiguousarray(wts).astype(bf16)
    bcv = (wm1 @ bproj + bmerge).astype(np.float32)

    # item shards: branch-major, original match order; core i owns
    # [512i, 512i+512).  Window extraction (the unfold) is pure relayout.
    b_all = np.concatenate([b_ids, b_ids])
    id_all = np.concatenate([l_ids, s_ids])
    h = (id_all // WO) * 4
    w = (id_all % WO) * 4
    fpad = np.stack([
        np.pad(f, ((0, 0), (0, 0), (2, 2), (2, 2))) for f in (f0, f1)
    ])                                              # [2, B, C, 244, 324]
    fpad = fpad.reshape(2 * B, C, 244, 324)
    mapid = np.repeat(np.arange(2), M) * B + b_all
    ki = np.arange(WINDOW)
    # windows[item, c, ki, kj] -> q = c*25 + ki*5 + kj (the torch scramble)
    wins = fpad[mapid[:, None, None, None],
                np.arange(C)[None, :, None, None],
                (h[:, None] + ki[None, :])[:, None, :, None],
                (w[:, None] + ki[None, :])[:, None, None, :]]
    wq = wins.reshape(2 * M, 25 * C).astype(bf16)   # [item, q]

    # coarse rows, pre-transposed to [k, item] per core
    cf = np.stack([c0, c1]).reshape(2 * B, L, DC)
    crows = cf[mapid, id_all].astype(bf16)          # [item, 256]

    in_maps = []
    for core in range(8):
        sl = slice(core * CAP, (core + 1) * CAP)
        # [d, (chunk, a, m)]
        tsd = np.ascontiguousarray(
            wq[sl].reshape(NCHUNK, GC, 25, 128).transpose(3, 0, 2, 1)
        ).reshape(128, NCHUNK * QD)
        # [k, (kchunk, item)]
        ctd = np.ascontiguousarray(
            crows[sl].reshape(CAP, 2, 128).transpose(2, 1, 0)
        ).reshape(128, 2 * CAP)
        in_maps.append({
            "tsd": tsd,
            "ctd": ctd,
            "wts": wts,
            "bcomb": bcv,
        })
    return in_maps


def _assemble(results):
    full = np.empty((2 * M, 25, 128), np.float32)
    for core, res in enumerate(results):
        og = np.asarray(res["out"]).reshape(128, NCHUNK, 25, GC)
        full[core * CAP:(core + 1) * CAP] = (
            og.transpose(1, 3, 2, 0).reshape(CAP, 25, 128).astype(np.float32))
    return full[:M], full[M:]


def _install_ntff_shim():
    """This image lacks ``antenv.axon_hooks``; recreate it so bass_utils'
    trace path can drive NTFF profiling via the axon PJRT .so."""
    import sys, types
    if "antenv.axon_hooks" in sys.modules:
        return
    import antenv  # noqa: F401
    mod = types.ModuleType("antenv.axon_hooks")
    mod._hook = None
    mod.set_axon_ntff_profile_hook = lambda h: setattr(mod, "_hook", h)
    mod.get_axon_ntff_profile_hook = lambda: mod._hook
    sys.modules["antenv.axon_hooks"] = mod
    try:
        from trn_agent_boot.trn_boot import _ntff_profile_via_ctypes
        mod._hook = _ntff_profile_via_ctypes("/opt/axon/libaxon_pjrt.so")
    except Exception:
        pass


def kernel(**inputs):
    from concourse import bass_utils

    in_maps = _host_prep(inputs)
    nc = _build_program()

    if os.environ.get("TRNK_SIM"):
        from concourse.bass_interp import CoreSim
        results = []
        ncore = int(os.environ.get("TRNK_SIM_CORES", "8"))
        for c in range(8):
            if c < ncore:
                sim = CoreSim(nc, trace=False)
                for name, val in in_maps[c].items():
                    sim.tensor(name)[:] = val
                sim.simulate()
                results.append({"out": np.array(sim.tensor("out"))})
            else:
                results.append({"out": np.zeros(128 * CAP * 25, np.float32)})
        return _assemble(results)

    trace = bool(os.environ.get("TRNK_TRACE"))
    kw = {}
    if trace:
        _install_ntff_shim()
        kw = dict(trace=True, trace_cores=list(range(8)))
    res = bass_utils.run_bass_kernel_spmd(nc, in_maps, core_ids=list(range(8)), **kw)
    if trace and res.exec_time_ns is not None:
        kernel.last_exec_time_ns = res.exec_time_ns
        kernel.last_mean_exec_time_ns = res.mean_exec_time_ns
        if res.instructions_and_trace:
            kernel.last_trace_path = res.instructions_and_trace[1]
    return _assemble(res.results)


kernel.last_exec_time_ns = None
kernel.last_mean_exec_time_ns = None
kernel.last_trace_path = None
